# revision 2
# baseline (speedup 1.0000x reference)
"""Multi-head self-attention (causal) for TRN2, 8 NeuronCores.

Sharding: core c handles batch b = c//2 and head-group g = c%2 (8 of 16
heads, i.e. feature columns [512g, 512g+512) of the QKV projections).
Each core computes its batch's attention output for its heads plus the
row-split output projection partial; the host sums the two partials per
batch (the pairwise all-reduce).

Per-core pipeline (all matmuls in float32r, 1 cycle/row on the PE):
  xT [1024,2048] (x^T, d on partitions) streams per 512-col s-block:
    Q^T[f,s], K^T[f,s] = w @ x^T   (features on partitions)
    V[s,f]             = x @ wv^T  (natural layout, +ones column -> V')
  attention per head in S^T layout (softmax without max-subtraction:
  scores are O(1) so exp is safe in fp32):
    S^T[sk,sq] = K_h @ Q_h^T       (dk=64 contraction)
    P^T = exp(S^T/8) -> bf16, causal mask on boundary tiles
    O'^T[65,sq] += V'_h^T @ P^T    (row 64 accumulates the denominator)
    O^T[f,s] = O'^T[0:64] * recip(O'^T[64]) broadcast
  out[s,1024] += O^T.T @ woT       (row-split partial, summed on host)
"""

import numpy as np
import ml_dtypes

_B, _S, _D = 4, 2048, 1024
_F = 512  # per-core feature columns (8 heads x dk=64)
_HPC = 8  # heads per core
_SB = 512  # s-block width
_NSB = _S // _SB  # 4
_NDT = _D // 128  # 8 contraction tiles for projections
_NCORES = 8

_nc_cache = None


def _build():
    import concourse.bacc as bacc
    import concourse.tile as tile
    from concourse import mybir

    F32 = mybir.dt.float32
    F32R = mybir.dt.float32r
    BF16 = mybir.dt.bfloat16
    AF = mybir.ActivationFunctionType

    nc = bacc.Bacc()
    xT = nc.dram_tensor("xT", [_D, _S], F32R, kind="ExternalInput")
    wqT = nc.dram_tensor("wqT", [_D, _F], F32R, kind="ExternalInput")
    wkT = nc.dram_tensor("wkT", [_D, _F], F32R, kind="ExternalInput")
    wvT = nc.dram_tensor("wvT", [_D, _F], F32R, kind="ExternalInput")
    woT = nc.dram_tensor("woT", [_F, _D], F32R, kind="ExternalInput")
    masks = nc.dram_tensor("masks", [128, 4, _SB], BF16, kind="ExternalInput")
    out = nc.dram_tensor("out", [_S, _D], F32, kind="ExternalOutput")

    with tile.TileContext(nc) as tc:
        with (
            tc.tile_pool(name="pers", bufs=1) as pers,
            tc.tile_pool(name="qpool", bufs=2) as qpool,
            tc.tile_pool(name="xpool", bufs=2) as xpool,
            tc.tile_pool(name="ppool", bufs=3) as ppool,
            tc.tile_pool(name="opool", bufs=1) as opool,
            tc.tile_pool(name="spool", bufs=2) as spool,
            tc.tile_pool(name="ps_mm", bufs=2, space="PSUM") as ps_mm,
            tc.tile_pool(name="ps_s", bufs=2, space="PSUM") as ps_s,
            tc.tile_pool(name="ps_o", bufs=2, space="PSUM") as ps_o,
        ):
            wq_s = pers.tile([128, _NDT, _F], F32R, tag="wq")
            wk_s = pers.tile([128, _NDT, _F], F32R, tag="wk")
            wv_s = pers.tile([128, _NDT, _F], F32R, tag="wv")
            nc.sync.dma_start(out=wq_s, in_=wqT.rearrange("(dt p) f -> p dt f", p=128))
            nc.sync.dma_start(out=wk_s, in_=wkT.rearrange("(dt p) f -> p dt f", p=128))
            nc.sync.dma_start(out=wv_s, in_=wvT.rearrange("(dt p) f -> p dt f", p=128))
            wo_s = pers.tile([128, 4, _D], F32R, tag="wo")
            nc.sync.dma_start(out=wo_s, in_=woT.rearrange("(ft p) d -> p ft d", p=128))
            mask_s = pers.tile([128, 4, _SB], BF16, tag="mask")
            nc.sync.dma_start(out=mask_s, in_=masks[:, :, :])

            # persistent K^T tiles: kts[ft][sbk] = K^T[128 f, 512 s]
            kts = [
                [pers.tile([128, _SB], F32R, tag=f"kt{ft}_{sbk}", name=f"kt{ft}_{sbk}") for sbk in range(_NSB)]
                for ft in range(4)
            ]
            # persistent V' tiles per s-tile of 128: [128 s, head, 65]
            vps = [pers.tile([128, _HPC, 65], BF16, tag=f"vp{st}", name=f"vp{st}") for st in range(16)]
            for st in range(16):
                nc.vector.memset(vps[st][:, :, 64:65], 1.0)

            xT_r = xT.rearrange("(dt p) s -> p dt s", p=128)

            for sb in range(_NSB):
                sq0 = sb * _SB
                xr = xpool.tile([128, _NDT, _SB], F32R, tag="xr")
                nc.sync.dma_start(out=xr, in_=xT_r[:, :, sq0 : sq0 + _SB])

                # ---- projections for this s-block ----
                qts = []
                for ft in range(4):
                    pq = ps_mm.tile([128, _SB], F32, tag="mm")
                    for dt_i in range(_NDT):
                        nc.tensor.matmul(
                            pq,
                            lhsT=wq_s[:, dt_i, ft * 128 : (ft + 1) * 128],
                            rhs=xr[:, dt_i, :],
                            start=(dt_i == 0),
                            stop=(dt_i == _NDT - 1),
                        )
                    qt = qpool.tile([128, _SB], F32R, tag=f"qt{ft}")
                    nc.vector.tensor_copy(qt, pq)
                    qts.append(qt)
                for ft in range(4):
                    pk = ps_mm.tile([128, _SB], F32, tag="mm")
                    for dt_i in range(_NDT):
                        nc.tensor.matmul(
                            pk,
                            lhsT=wk_s[:, dt_i, ft * 128 : (ft + 1) * 128],
                            rhs=xr[:, dt_i, :],
                            start=(dt_i == 0),
                            stop=(dt_i == _NDT - 1),
                        )
                    nc.vector.tensor_copy(kts[ft][sb], pk)
                for stl in range(4):
                    st = sb * 4 + stl
                    pv = ps_mm.tile([128, _SB], F32, tag="mm")
                    for dt_i in range(_NDT):
                        nc.tensor.matmul(
                            pv,
                            lhsT=xr[:, dt_i, stl * 128 : (stl + 1) * 128],
                            rhs=wv_s[:, dt_i, :],
                            start=(dt_i == 0),
                            stop=(dt_i == _NDT - 1),
                        )
                    nc.scalar.activation(
                        vps[st][:, :, 0:64],
                        pv.rearrange("p (h d) -> p h d", h=_HPC),
                        AF.Copy,
                    )

                # ---- attention for sq-block sb, all heads ----
                n_sk = 4 * (sb + 1)
                n_g = n_sk // 2
                ots = [opool.tile([128, _SB], F32R, tag=f"ot{ft}", name=f"ot{ft}") for ft in range(4)]
                for h in range(_HPC):
                    ft_h, oh = h // 2, h % 2
                    qt_h = qts[ft_h][oh * 64 : (oh + 1) * 64, :]
                    po = ps_o.tile([65, _SB], F32, tag="o")
                    for gi in range(n_g):
                        sps = ps_s.tile([128, 2, _SB], F32, tag="s")
                        for u in range(2):
                            t = 2 * gi + u
                            lhs = kts[ft_h][t // 4][
                                oh * 64 : (oh + 1) * 64,
                                (t % 4) * 128 : (t % 4 + 1) * 128,
                            ]
                            nc.tensor.matmul(
                                sps[:, u, :], lhsT=lhs, rhs=qt_h, start=True, stop=True
                            )
                        pt = ppool.tile([128, 2, _SB], BF16, tag="pt")
                        nc.scalar.activation(pt, sps, AF.Exp, scale=0.125)
                        if 2 * gi >= 4 * sb:  # boundary group: causal mask
                            m0 = 2 * gi - 4 * sb
                            nc.vector.tensor_mul(pt, pt, mask_s[:, m0 : m0 + 2, :])
                        for u in range(2):
                            t = 2 * gi + u
                            nc.tensor.matmul(
                                po,
                                lhsT=vps[t][:, h, :],
                                rhs=pt[:, u, :],
                                start=(t == 0),
                                stop=(t == n_sk - 1),
                            )
                    dinv = spool.tile([1, _SB], F32, tag="dinv")
                    nc.vector.reciprocal(dinv, po[64:65, :])
                    dbc = spool.tile([64, _SB], F32, tag="dbc")
                    nc.gpsimd.partition_broadcast(dbc[:, :], dinv[0:1, :])
                    nc.vector.tensor_mul(
                        ots[ft_h][oh * 64 : (oh + 1) * 64, :], po[0:64, :], dbc[:, :]
                    )

                # ---- output projection for this s-block ----
                for stl in range(4):
                    for db in range(2):
                        pp = ps_mm.tile([128, _SB], F32, tag="mm")
                        for ft in range(4):
                            nc.tensor.matmul(
                                pp,
                                lhsT=ots[ft][:, stl * 128 : (stl + 1) * 128],
                                rhs=wo_s[:, ft, db * _SB : (db + 1) * _SB],
                                start=(ft == 0),
                                stop=(ft == 3),
                            )
                        ost = spool.tile([128, _SB], F32, tag="ost")
                        nc.scalar.activation(ost, pp, AF.Copy)
                        nc.sync.dma_start(
                            out=out[
                                sq0 + stl * 128 : sq0 + (stl + 1) * 128,
                                db * _SB : (db + 1) * _SB,
                            ],
                            in_=ost,
                        )

    nc.finalize()
    return nc


def _make_masks():
    i = np.arange(128)[:, None, None]
    m = np.arange(4)[None, :, None]
    j = np.arange(_SB)[None, None, :]
    return ((i + 128 * m) <= j).astype(ml_dtypes.bfloat16)


def kernel(x, wq, wk, wv, wo):
    global _nc_cache
    from concourse.bass_utils import run_bass_kernel_spmd

    x = np.asarray(x, dtype=np.float32)
    wq = np.asarray(wq, dtype=np.float32)
    wk = np.asarray(wk, dtype=np.float32)
    wv = np.asarray(wv, dtype=np.float32)
    wo = np.asarray(wo, dtype=np.float32)

    if _nc_cache is None:
        _nc_cache = _build()
    nc = _nc_cache

    masks_np = _make_masks()
    in_maps = []
    for c in range(_NCORES):
        b, g = c // 2, c % 2
        cols = slice(g * _F, (g + 1) * _F)
        in_maps.append(
            {
                "xT": np.ascontiguousarray(x[b].T),
                "wqT": np.ascontiguousarray(wq[cols, :].T),
                "wkT": np.ascontiguousarray(wk[cols, :].T),
                "wvT": np.ascontiguousarray(wv[cols, :].T),
                "woT": np.ascontiguousarray(wo[:, cols].T),
                "masks": masks_np,
            }
        )

    res = run_bass_kernel_spmd(nc, in_maps, list(range(_NCORES)), trace=False)
    outs = [r["out"] for r in res.results]
    y = np.stack([outs[2 * b] + outs[2 * b + 1] for b in range(_B)])
    return y.astype(np.float32)
